# revision 2
# baseline (speedup 1.0000x reference)
"""Trainium2 Bass kernel: per-voxel eigenvalues of 3x3 symmetric matrices.

Input  X: (2, 9, 96, 96, 96) float32 -- each voxel holds a row-major 3x3
matrix in the channel dim.  Output: (2, 3, 96, 96, 96) float32, eigenvalues
ascending in the channel dim.

v2 changes vs v1 (93.4 us/rep steady state, Vector-bound at ~76 us busy):
  - fp16 compute: inputs are cast fp32->fp16 in-flight by gpsimd (software
    DGE) DMA; DVE adds/subs run in 2x mode (~0.98 us @1728 vs 1.88 fp32),
    tensor_scalar in 4x mode (~0.53 us).  Multiplies are 1x at any dtype.
  - one chunk of free=1728 per rep (amortizes per-instruction overhead).
  - asin via direct tangent form: asin(r) = atan(r / sqrt(1-r^2)) with
    1/sqrt computed as exp(-0.5 ln(1-r^2)): drops 2 ACT ops vs the
    half-angle form.
  - lmid = tr - lmax - lmin (saves one 2p*sin multiply + one Sin ACT).
  - det = (y1*0.25 via 4x tensor_scalar) + G1 (2x add).
  - squares split S/Pool to unload DVE; fp16 stores (host upcasts).
  - fp32 kept where fp16 would overflow or cancel: e1 = 1/(2p^3) (up to
    ~1e6), lnp2/ln1mr2 (log precision), r2/r2c (clamp granularity near 1).

Engine budget per rep @1728 (est): V ~39us, S ~22us, G ~17us, DMA ~27us.
"""

import sys

if "/opt/trn_rl_repo" not in sys.path:
    sys.path.insert(0, "/opt/trn_rl_repo")

import math

import numpy as np

N_CORES = 8
B = 2
DHW = 96 * 96 * 96          # 884736 voxels per batch
PER = DHW // N_CORES        # 110592 voxels per batch per core
P = 128                     # SBUF partitions
FB = PER // P               # 864 free elems per batch per core
FT = B * FB                 # 1728: packed free dim per core (both batches)

SQRT2 = math.sqrt(2.0)
E1_BIAS = 0.5 * math.log(54.0) + 1.5 * math.log(2.0)
P2_BIAS = -0.5 * math.log(3.0)
LN_EPS = 1e-20
TWO_PI_3 = 2.0 * math.pi / 3.0
PI_3 = math.pi / 3.0
R2_CLAMP = 1.0 - 2.0 ** -23
PERM = [0, 4, 8, 1, 3, 2, 6, 5, 7]   # channel order in the DRAM layout

_CACHE = {}


def _build(split_waits=True, nrep=1):
    import concourse.bass as bass
    import concourse.tile as tile
    from concourse import mybir

    fp32 = mybir.dt.float32
    fp16 = mybir.dt.float16
    AF = mybir.ActivationFunctionType
    ALU = mybir.AluOpType

    nc = bass.Bass("TRN2", target_bir_lowering=False, debug=False,
                   num_devices=N_CORES)
    x = nc.dram_tensor("x", [P, 9, FT], fp32, kind="ExternalInput").ap()
    y = nc.dram_tensor("y", [3, P, FT], fp16, kind="ExternalOutput").ap()

    # Activation biases must exist as SBUF const APs before use.
    for cval in (E1_BIAS, P2_BIAS, LN_EPS, TWO_PI_3, PI_3, 1.0):
        cval = float(cval)
        if (fp32, cval) not in nc.const_aps.aps:
            ctens = nc.alloc_sbuf_tensor(f"const-f32-{cval}", [128, 1], fp32)
            nc.gpsimd.memset(ctens.ap(), cval)
            nc.const_aps.aps[(fp32, cval)] = ctens.ap()
    nc.all_engine_barrier()

    V, G, S = nc.vector, nc.gpsimd, nc.scalar

    with tile.TileContext(nc) as tc:
        with tc.tile_pool(name="sl", bufs=1) as pool:
            N16 = 14                 # fp16 slots per parity (plus 3 group tiles)
            N32 = 2                  # fp32 slots per parity
            name2slot = {}
            tiles = {}
            free16 = []
            free32 = []

            def alloc(name, dt=fp16):
                if dt == fp16:
                    s = free16.pop(0)
                    tag = f"h{s}"
                else:
                    s = free32.pop(0)
                    tag = f"w{s}"
                name2slot[name] = (dt, s)
                t = pool.tile([P, FT], dt, tag=tag, name=f"t_{tag}_{name}")
                tiles[name] = t
                return t

            def ap(name):
                return tiles[name][:, :]

            def rel(*names):
                for name in names:
                    dt, s = name2slot.pop(name)
                    (free16 if dt == fp16 else free32).append(s)
                    del tiles[name]

            def tt(eng, dst, a, b, op, dt=fp16):
                alloc(dst, dt)
                fn = {"add": eng.tensor_add, "sub": eng.tensor_sub,
                      "mul": eng.tensor_mul}[op]
                fn(ap(dst), ap(a), ap(b))

            act_insts = {}
            cur_rep = [0]

            def act(dst, src, func, scale=1.0, bias=0.0, dt=fp16):
                alloc(dst, dt)
                inst = S.activation(ap(dst), ap(src), func,
                                    bias=float(bias), scale=float(scale))
                act_insts[(cur_rep[0], dst)] = inst

            def emit_loads(rep):
                # 9 channel planes as 3 grouped cast DMAs (fp32->fp16
                # in-flight; 3 Pool issues).  Issued one rep ahead so the
                # transfers overlap the previous rep's compute.
                lpar = rep % 2
                for g in range(3):
                    t = pool.tile([P, 3 * FT], fp16, tag=f"g{lpar}{g}",
                                  name=f"xg{lpar}{g}_{rep}")
                    tiles[f"g{g}@{lpar}"] = t
                    G.dma_start(out=t[:, :], in_=x[:, 3 * g:3 * g + 3, :])

            for rep in range(nrep):
                cur_rep[0] = rep
                par = rep % 2
                free16[:] = [par * N16 + s for s in range(N16)]
                free32[:] = [par * N32 + s for s in range(N32)]
                emit_loads(rep)

                # host-side channel permutation: g0=diag(0,4,8), g1=(1,3,2),
                # g2=(6,5,7) -- the q/aq/bq/cq front runs off g0 alone
                def xch(ch):
                    pos = PERM.index(ch)
                    g, o = divmod(pos, 3)
                    return tiles[f"g{g}@{par}"][:, o * FT:(o + 1) * FT]

                # ---- linear stage (all fp16, DVE 2x adds); g0 first
                alloc("t0")
                V.tensor_add(ap("t0"), xch(0), xch(4))
                alloc("tr")
                V.tensor_add(ap("tr"), ap("t0"), xch(8))
                rel("t0")
                alloc("q")
                V.tensor_scalar_mul(ap("q"), ap("tr"), 1.0 / 3.0)
                alloc("aq")
                V.tensor_sub(ap("aq"), xch(0), ap("q"))
                alloc("bq")
                V.tensor_sub(ap("bq"), xch(4), ap("q"))
                alloc("cq")
                V.tensor_sub(ap("cq"), xch(8), ap("q"))
                # deviatoric diag squares (pre-doubled via scale sqrt(2))
                act("a2", "aq", AF.Square, scale=SQRT2)
                act("b2", "bq", AF.Square, scale=SQRT2)
                act("c2", "cq", AF.Square, scale=SQRT2)
                tt(V, "s2", "a2", "b2", "add")
                rel("a2", "b2")
                tt(V, "s3", "s2", "c2", "add")
                rel("s2", "c2")
                # off-diagonal doubled entries (g1, then g2)
                alloc("D")
                V.tensor_add(ap("D"), xch(1), xch(3))
                act("ddq", "D", AF.Square)
                alloc("E")
                V.tensor_add(ap("E"), xch(2), xch(6))
                alloc("F")
                V.tensor_add(ap("F"), xch(5), xch(7))
                del tiles[f"g0@{par}"], tiles[f"g1@{par}"], tiles[f"g2@{par}"]
                act("eeq", "E", AF.Square)
                act("ffq", "F", AF.Square)
                tt(V, "s1", "ddq", "eeq", "add")
                tt(V, "p1", "s1", "ffq", "add")
                rel("s1")
                tt(V, "p2x", "s3", "p1", "add")
                rel("s3", "p1")

                # ---- det(A - qI)
                tt(V, "w1", "D", "E", "mul")
                tt(V, "w2", "w1", "F", "mul")
                rel("D", "E", "F", "w1")
                tt(V, "u1", "bq", "cq", "mul")
                tt(V, "G1", "aq", "u1", "mul")
                rel("u1")
                tt(V, "v1", "aq", "ffq", "mul")
                tt(V, "v2", "cq", "ddq", "mul")
                tt(V, "v3", "bq", "eeq", "mul")
                rel("aq", "bq", "cq", "ddq", "eeq", "ffq")
                tt(V, "v4", "v1", "v2", "add")
                rel("v1", "v2")
                tt(V, "v5", "v4", "v3", "add")
                rel("v4", "v3")
                tt(V, "y1", "w2", "v5", "sub")
                rel("w2", "v5")
                alloc("y1q")
                V.tensor_scalar_mul(ap("y1q"), ap("y1"), 0.25)
                rel("y1")
                tt(V, "det", "G1", "y1q", "add")
                rel("G1", "y1q")

                # ---- r = det/(2p^3) via ln/exp (table: natural_log_exp)
                act("lnp2", "p2x", AF.Ln, bias=LN_EPS, dt=fp32)
                rel("p2x")
                act("e1", "lnp2", AF.Exp, scale=-1.5, bias=E1_BIAS, dt=fp32)
                act("P2", "lnp2", AF.Exp, scale=0.5, bias=P2_BIAS)   # 2p
                rel("lnp2")
                tt(V, "rr", "det", "e1", "mul")                      # r
                rel("det", "e1")
                act("r2", "rr", AF.Square, dt=fp32)
                alloc("r2c", fp32)
                V.tensor_scalar_min(ap("r2c"), ap("r2"), R2_CLAMP)
                rel("r2")
                act("ln1mr2", "r2c", AF.Ln, scale=-1.0, bias=1.0, dt=fp32)
                rel("r2c")
                act("w", "ln1mr2", AF.Exp, scale=-0.5)   # 1/sqrt(1-r^2)
                rel("ln1mr2")
                tt(V, "t2", "rr", "w", "mul")            # tan(asin(r))
                rel("rr", "w")
                # table switch -> trig_and_small
                act("at", "t2", AF.Arctan, dt=fp32)      # asin(r)
                rel("t2")
                act("c1", "at", AF.Sin, scale=-1.0 / 3.0, bias=TWO_PI_3)
                act("c2n", "at", AF.Sin, scale=-1.0 / 3.0, bias=PI_3)
                rel("at")

                # ---- eigenvalues: lmax = q + 2p*c1, lmin = q - 2p*c2n,
                #      lmid = tr - lmax - lmin
                tt(V, "m1", "P2", "c1", "mul")
                tt(V, "lmax", "q", "m1", "add")
                rel("c1", "m1")
                tt(V, "m2", "P2", "c2n", "mul")
                tt(V, "lmin", "q", "m2", "sub")
                rel("c2n", "m2", "P2", "q")
                tt(V, "t3", "tr", "lmax", "sub")
                rel("tr")
                tt(V, "lmid", "t3", "lmin", "sub")
                rel("t3")

                # ---- store ascending eigenvalues (fp16)
                for k, name in enumerate(("lmin", "lmid", "lmax")):
                    nc.sync.dma_start(out=y[k], in_=ap(name))
                rel("lmin", "lmid", "lmax")

    if split_waits:
        _split_multi_waits(nc, mybir)
    return nc


def _split_multi_waits(nc, mybir):
    """walrus codegen allows a single sync-wait slot per TPB instruction;
    hoist extra waits onto standalone NoOps on the same engine."""
    for f in nc.m.functions:
        for blk in f.blocks:
            il = blk.instructions
            i = 0
            while i < len(il):
                inst = il[i]
                si = inst.sync_info
                if si is not None and si.on_wait and len(si.on_wait) > 1:
                    waits = list(si.on_wait)
                    for w in waits[:-1]:
                        nop = mybir.InstNoOp(
                            name=nc.get_next_instruction_name(),
                            engine=inst.engine,
                            ins=[],
                            outs=[],
                            sync_info=mybir.SyncInfo(on_wait=[w], on_update=[]),
                            bass_nofuse=True,
                        )
                        il.insert(i, nop)
                        i += 1
                    si.on_wait = waits[-1:]
                i += 1


def get_program():
    if "nc" not in _CACHE:
        _CACHE["nc"] = _build()
    return _CACHE["nc"]


def shard_inputs(X):
    """X: (2,9,96,96,96) float32 -> list of per-core {"x": (128,9,1728)}."""
    x = np.asarray(X, dtype=np.float32).reshape(B, 9, DHW)
    maps = []
    for c in range(N_CORES):
        slab = x[:, :, c * PER:(c + 1) * PER].reshape(B, 9, P, FB)
        # (B,9,P,FB) -> (P,9,B,FB) -> (P,9,FT): per-partition rows hold all
        # nine channels contiguously, so grouped cast DMAs read 20.7 KB runs
        xc = np.ascontiguousarray(
            slab[:, PERM].transpose(2, 1, 0, 3)).reshape(P, 9, FT)
        maps.append({"x": xc})
    return maps


def unshard_outputs(results):
    out = np.empty((B, 3, DHW), dtype=np.float32)
    for c, r in enumerate(results):
        yc = np.asarray(r["y"]).astype(np.float32)
        yc = yc.reshape(3, P, B, FB).transpose(2, 0, 1, 3)
        out[:, :, c * PER:(c + 1) * PER] = yc.reshape(B, 3, PER)
    return out.reshape(B, 3, 96, 96, 96)


def kernel(X):
    from concourse.bass_utils import run_bass_kernel_spmd

    nc = get_program()
    in_maps = shard_inputs(np.asarray(X))
    res = run_bass_kernel_spmd(nc, in_maps, list(range(N_CORES)))
    return unshard_outputs(res.results)
